# revision 47
# baseline (speedup 1.0000x reference)
"""Trainium2 Bass kernel for nn_MultiHeadAttention_60816736911814.

Reference semantics (all derived from `src`; `k`/`v` args ignored):
  x  = channel_shuffle(src)          # [B,S,G,C]->[B,S,C,G] flatten, G=5
  xh = split_heads(x)                # [B,H,S,dk], H=16, dk=80
  q/k/v = per-head Linear(dk,dk)     # weights [H,dk,dk] + bias
  attn  = softmax(q k^T / sqrt(dk)) v
  out   = concat(attn) @ Wo^T + bo   # Wo [D,D], D=1280

Sharding (8 cores, no collectives): core i handles batch b=i//2 and query
rows [512*(i%2), +512). Each core gets src[b] ROLLED so its query rows are
rows 0..511 (key order is irrelevant to softmax+sum); all cores run an
identical program and the full output is a pure concatenation.

Device-side algebraic restructuring (all folds are host-side, layout-only
or tiny weight-matrix products):
 - K projection eliminated: scores = x~_k^T Mt x~_q with Mt = Wq_aug @
   Wk_aug^T per head (x~ = [x;1] handles both biases). One projection
   (qq = Mt^T x~_q) replaces q AND k projections.
 - V projection and output projection fused: out = sum_h Rn_h^T G_h where
   R_h = XS_h P_h ([1+dk, SH], row 0 = Z = softmax denominator via the
   ones-column of XS), Rn = R * bcast(1/Z), and G_h = Wv_aug_h @
   Wo_h_block^T (+ bo folded into head 0's row 0, whose Rn value is 1).
 - Channel shuffle / head split / transposes are folded into the host-side
   layouts of xh (d-major) and xs (s-major): no on-device transposes.

Engine schedule: ACT does only the 48 exp instructions (scores land in
2/3-bank fused PSUM tiles so each Exp covers 1536/1024 columns); casts run
on GpSimd, normalization mul + reciprocal on DVE, 1/Z partition-broadcast
via SBUF->SBUF DMA. The PE stream is software-pipelined (scores(h) ->
qq(h+1) -> R(h-1)) so the Tensor engine stays continuously busy (max
p-state clock); qq and R share one rotating 2-slot PSUM tag so everything
fits the 8 PSUM banks alongside the 6 score banks.
"""

import numpy as np
import ml_dtypes

B, S, D = 4, 1024, 1280
H, DK, G = 16, 80, 5
N_CORES = 8
SH = S // 2  # 512 query rows per core
SCALE = 1.0 / float(np.sqrt(DK))
NT = S // 128  # 8 k-tiles
DA = DK + 1  # 81: augmented channel dim
NJ = H * DA  # 1296 packed (h,d) rows
NPL = (NJ + 127) // 128  # 11 planes
# per-head score fusion: k-tiles per fused PSUM tile / exp instruction
FUSE = [3, 3, 2]

_BUILT = {}


def _legalize_waits(nc, mybir):
    """This walrus build allows 1 sync-wait per instruction (2 on
    EventSemaphore). Tile can emit more; split overflow waits onto
    injected same-engine NoOp carriers placed just before the
    instruction (engines run their stream in order -> AND semantics)."""
    n_fix = 0
    for f in nc.m.functions:
        for blk in f.blocks:
            out = []
            changed = False
            for inst in blk.instructions:
                cap = 2 if type(inst).__name__ == "InstEventSemaphore" else 1
                si = inst.sync_info
                if si is not None and si.on_wait and len(si.on_wait) > cap:
                    waits = list(si.on_wait)
                    for w in waits[:-cap]:
                        nop = mybir.InstNoOp(name=f"I-waitfix-{n_fix}")
                        n_fix += 1
                        nop.engine = inst.engine
                        nop.sync_info = mybir.SyncInfo(on_wait=[w], on_update=[])
                        out.append(nop)
                    inst.sync_info = mybir.SyncInfo(
                        on_wait=waits[-cap:], on_update=list(si.on_update)
                    )
                    changed = True
                out.append(inst)
            if changed:
                try:
                    blk.instructions = out
                except Exception:
                    blk.instructions.clear()
                    blk.instructions.extend(out)
    return n_fix


def _ldw_peephole(nc):
    """Drop redundant weight reloads: a matmul whose stationary operand is
    byte-identical to the immediately preceding PE matmul's keeps the loaded
    weights (ldweights=False). Only Gproj's 3-way ocut reuse and the preheat
    chain match."""
    n = 0
    for f in nc.m.functions:
        for blk in f.blocks:
            prev = None
            for inst in blk.instructions:
                if type(inst).__name__ != "InstMatmult":
                    continue
                w = inst.ins[1]
                key = (
                    getattr(w, "memref", None),
                    getattr(w, "offset", None),
                    str(getattr(w, "ap", None)),
                    str(inst.is_transpose),
                    str(inst.perf_mode),
                )
                if prev == key:
                    inst.ldweights = False
                    n += 1
                prev = key
    return n


def _build(legalize=True):
    import os as _os
    import concourse.bass as bass
    import concourse.mybir as mybir
    import concourse.tile as tile

    _fuse = FUSE

    f32 = mybir.dt.float32
    bf16 = mybir.dt.bfloat16

    nc = bass.Bass(trn_type="TRN2", target_bir_lowering=False, debug=False)

    # xh[d, h, s]: d-major shuffled x, row 80 = ones
    xh_d = nc.dram_tensor("xh", [DA, H, S], bf16, kind="ExternalInput").ap()
    # xs[p, h, t, j]: s-major shuffled x (k = t*128+p), col j=0 = ones
    xs_d = nc.dram_tensor("xs", [128, H, NT, DA], bf16, kind="ExternalInput").ap()
    # mt[d1, h, d2] = (Wq_aug @ Wk_aug^T)[d1, d2] per head
    mt_d = nc.dram_tensor("mt", [DA, H, DA], bf16, kind="ExternalInput").ap()
    # g[p, pl, o]: packed rows j = 81h + d of G_h = Wv_aug @ Wo_h^T (+bo)
    g_d = nc.dram_tensor("g", [128, NPL, D], bf16, kind="ExternalInput").ap()
    out_d = nc.dram_tensor("out", [SH, D], f32, kind="ExternalOutput").ap()

    with tile.TileContext(nc) as tc:
        with (
            tc.tile_pool(name="const", bufs=1) as const,
            tc.tile_pool(name="big", bufs=1) as big,
            tc.tile_pool(name="et", bufs=6) as etp,
            tc.tile_pool(name="sm", bufs=2) as sm,
            tc.tile_pool(name="ps", bufs=2, space="PSUM") as ps,
        ):
            on2 = const.tile([128, 512], bf16)
            nc.gpsimd.memset(on2, 1.0)

            mt_sb = big.tile([DA, H, DA], bf16)
            xh_sb = big.tile([DA, H, S], bf16)
            xs_sb = big.tile([128, H, NT, DA], bf16)
            g_sb = big.tile([128, NPL, D], bf16)
            ct = big.tile([DA, H, SH], bf16)  # normalized heads, h-major
            ctp = big.tile([128, NPL, SH], bf16)  # packed rows j = 81h+d

            # --- input DMA: demand-driven. Only what iterations 0..3 need is
            # loaded upfront; the rest is prefetched per-head inside the loop
            # so no single bulk transfer blocks startup. ACT issues no DMAs
            # (DMA transfer time occupies the issuing engine). ---
            nc.sync.dma_start(out=mt_sb, in_=mt_d)
            nc.sync.dma_start(out=xh_sb[:, 0:2, :], in_=xh_d[:, 0:2, :])
            nc.gpsimd.dma_start(out=xs_sb[:, 0:2, :, :], in_=xs_d[:, 0:2, :, :])
            nc.sync.dma_start(out=xh_sb[:, 2:4, :], in_=xh_d[:, 2:4, :])

            # --- PE p-state preheat: dummy matmuls so the clock is ramped
            # when real work starts ---
            for i in range(9):
                ph = ps.tile([DA, 512], f32, tag="qr", bufs=2, name="ph")
                nc.tensor.matmul(
                    ph[0:1, :], on2[0:1, 0:1], on2[0:1, :], start=True, stop=True
                )

            qq_sbs = {}
            ets = {}
            r_pss = {}
            hus = {}
            # head groups for softmax normalization; the last groups are
            # small so the final normalization chains are short
            HGROUPS = [
                (0, 1, 2, 3), (4, 5, 6, 7), (8, 9, 10, 11), (12, 13), (14,), (15,),
            ]
            GOF = {h: (gi, k) for gi, g in enumerate(HGROUPS) for k, h in enumerate(g)}
            zgs = {}
            norm_st = {}

            fetched = {("xh", 0), ("xh", 1), ("xh", 2), ("xh", 3),
                       ("xs", 0), ("xs", 1)}

            def prefetch(kind, h):
                if h > H - 1 or (kind, h) in fetched:
                    return
                fetched.add((kind, h))
                if kind == "xh":
                    nc.sync.dma_start(
                        out=xh_sb[:, h : h + 1, :], in_=xh_d[:, h : h + 1, :]
                    )
                else:
                    nc.gpsimd.dma_start(
                        out=xs_sb[:, h : h + 1, :, :], in_=xs_d[:, h : h + 1, :, :]
                    )

            def issue_qq(h):
                prefetch("xh", h + 3)
                prefetch("xs", h + 1)
                qq_ps = ps.tile([DA, 512], f32, tag="qr", bufs=2, name="qq_ps")
                nc.tensor.matmul(
                    qq_ps, mt_sb[:, h, :], xh_sb[:, h, 0:SH], start=True, stop=True
                )
                qq_sb = sm.tile([DA, 512], bf16, tag="qq_sb", bufs=2, name="qq_sb")
                nc.vector.tensor_copy(qq_sb, qq_ps)
                qq_sbs[h] = qq_sb

            def issue_scores(h):
                if 4 <= h <= 8:
                    j = 2 * (h - 4)
                    eng = nc.sync if h % 2 == 0 else nc.gpsimd
                    eng.dma_start(
                        out=g_sb[:, j : j + 2, :], in_=g_d[:, j : j + 2, :]
                    )
                elif h == 9:
                    nc.sync.dma_start(
                        out=g_sb[:, 10:11, :], in_=g_d[:, 10:11, :]
                    )
                qq_sb = qq_sbs.pop(h)
                ets[h] = []
                kt = 0
                for nf in _fuse:
                    sc = ps.tile([128, nf * 512], f32, tag="sc", bufs=2, name="sc")
                    for u in range(nf):
                        nc.tensor.matmul(
                            sc[:, u * 512 : u * 512 + 512],
                            xh_sb[:, h, (kt + u) * 128 : (kt + u) * 128 + 128],
                            qq_sb,
                            start=True,
                            stop=True,
                        )
                    kt += nf
                    et = etp.tile(
                        [128, nf * 512], bf16, tag="et", name="et",
                        bufs=3 * len(_fuse),
                    )
                    nc.scalar.activation(
                        et[:, 0 : nf * 512],
                        sc[:, 0 : nf * 512],
                        mybir.ActivationFunctionType.Exp,
                        scale=SCALE,
                    )
                    ets[h].append(et)

            def issue_r(h):
                r_ps = ps.tile([DA, 512], f32, tag="qr", bufs=2, name="r_ps")
                kt = 0
                for f, nf in enumerate(_fuse):
                    for u in range(nf):
                        nc.tensor.matmul(
                            r_ps,
                            xs_sb[:, h, kt + u, :],
                            ets[h][f][:, u * 512 : u * 512 + 512],
                            start=(kt + u == 0),
                            stop=(kt + u == NT - 1),
                        )
                    kt += nf
                del ets[h]
                gi, k = GOF[h]
                if len(HGROUPS[gi]) > 1:
                    if k == 0:
                        zgs[gi] = sm.tile(
                            [128, 512], f32, tag="zg", bufs=2, name="zg"
                        )
                        nc.gpsimd.memset(zgs[gi], 1.0)
                    nc.scalar.copy(
                        zgs[gi][32 * k : 32 * k + 1, :], r_ps[0:1, :]
                    )
                else:
                    r_pss[h] = r_ps  # single-head group: recip reads PSUM row
                hu = sm.tile([DA, 512], bf16, tag="hu", bufs=8, name="hu")
                nc.vector.tensor_copy(hu, r_ps)
                hus[h] = hu

            def norm_a(gi, nch=1):
                """Reciprocal of Z + broadcast DMAs for group gi. nch>1
                pipelines the chain in q-tile chunks (tail latency)."""
                nr = 32 * (len(HGROUPS[gi]) - 1) + 1
                zr = sm.tile([128, 512], f32, tag="zr", bufs=2, name="zr")
                zin = zgs.pop(gi) if len(HGROUPS[gi]) > 1 else r_pss.pop(
                    HGROUPS[gi][0]
                )
                w = 512 // nch
                for c in range(nch):
                    nc.vector.reciprocal(
                        zr[0:nr, c * w : c * w + w], zin[0:nr, c * w : c * w + w]
                    )
                zrb = sm.tile([128, 512], bf16, tag="zrb", bufs=2, name="zrb")
                for c in range(nch):
                    nc.vector.tensor_copy(
                        zrb[0:nr, c * w : c * w + w], zr[0:nr, c * w : c * w + w]
                    )
                zbcs = []
                for k in range(len(HGROUPS[gi])):
                    zbc = sm.tile([DA, 512], bf16, tag="zbc", bufs=8, name="zbc")
                    # partition-broadcast via zero-stride middle dim: the DMA
                    # reads the same line DA times; split across both DMA
                    # queues (the 81x re-read is bandwidth-bound)
                    h2 = 256 // nch
                    for c in range(nch):
                        for eng, c0 in ((nc.sync, 2 * c * h2), (nc.gpsimd, 2 * c * h2 + h2)):
                            src = zrb[32 * k : 32 * k + 1, c0 : c0 + h2]
                            src_b = bass.AP(
                                src.tensor, src.offset,
                                [[512, 1], [0, DA], [1, h2]],
                            )
                            eng.dma_start(out=zbc[:, c0 : c0 + h2], in_=src_b)
                    zbcs.append(zbc)
                norm_st[gi] = zbcs

            def norm_b(gi, nch=1):
                """Normalize + repack. In-phase groups multiply on Pool
                (all-SBUF, keeps DVE clear); tail groups on DVE chunked."""
                zbcs = norm_st.pop(gi)
                eng_mul = nc.vector.tensor_mul if gi >= 3 else nc.gpsimd.tensor_mul
                w = 512 // nch
                for k, hh in enumerate(HGROUPS[gi]):
                    hu = hus.pop(hh)
                    j0 = DA * hh
                    pl, off = j0 // 128, j0 % 128
                    l1 = min(128 - off, DA)
                    for c in range(nch):
                        s = slice(c * w, c * w + w)
                        eng_mul(ct[:, hh, s], hu[:, s], zbcs[k][:, s])
                        nc.gpsimd.dma_start(
                            out=ctp[off : off + l1, pl, s], in_=ct[0:l1, hh, s]
                        )
                        if l1 < DA:
                            nc.sync.dma_start(
                                out=ctp[0 : DA - l1, pl + 1, s],
                                in_=ct[l1:DA, hh, s],
                            )

            # --- head phase, software pipelined: PE runs scores(h), qq(h+1),
            # R(h-2); normalization trails by group, spread over two
            # iterations (reciprocal is slow on DVE) and issued after the qq
            # cast so it never delays the next head's scores ---
            issue_qq(0)
            a_due, b_due = [], []
            for h in range(H):
                issue_scores(h)
                if h + 1 < H:
                    issue_qq(h + 1)
                if b_due:
                    norm_b(b_due.pop(0))
                if a_due:
                    gi = a_due.pop(0)
                    norm_a(gi)
                    b_due.append(gi)
                if h >= 2:
                    hh = h - 2
                    issue_r(hh)
                    gi = GOF[hh][0]
                    if hh == HGROUPS[gi][-1] and hh <= 11:
                        a_due.append(gi)

            # --- output projection: out[q, o] = sum_j ctp[j, q] g[j, o].
            # Chunk-outer per q-tile: the three output-column accumulators
            # share each ctp stationary load (the ldweights peephole drops
            # the redundant reloads). ---
            OCUTS = [(0, 512), (512, 1024), (1024, 1280)]
            qt_ops = {}

            def gproj_qt(qt, j_lo, j_hi, tags=("sc", "sc", "qr")):
                if qt not in qt_ops:
                    qt_ops[qt] = [
                        ps.tile([128, 512], f32, tag=tags[c], bufs=2, name="op")
                        for c in range(3)
                    ]
                for j in range(j_lo, j_hi):
                    kh = 128 if j < NPL - 1 else NJ - 128 * (NPL - 1)
                    for c, (o0, o1) in enumerate(OCUTS):
                        nc.tensor.matmul(
                            qt_ops[qt][c][:, 0 : o1 - o0],
                            ctp[0:kh, j, qt * 128 : qt * 128 + 128],
                            g_sb[0:kh, j, o0:o1],
                            start=(j == 0),
                            stop=(j == NPL - 1),
                        )
                if j_hi == NPL:
                    for c, (o0, o1) in enumerate(OCUTS):
                        o_sb = sm.tile(
                            [128, 512], f32, tag="osb", bufs=2, name="o_sb"
                        )
                        nc.vector.tensor_copy(
                            o_sb[:, 0 : o1 - o0], qt_ops[qt][c][:, 0 : o1 - o0]
                        )
                        nc.gpsimd.dma_start(
                            out=out_d[qt * 128 : qt * 128 + 128, o0:o1],
                            in_=o_sb[:, 0 : o1 - o0],
                        )
                    del qt_ops[qt]

            # plane deps: 0..7 <- heads <= 13; 8 <- head 14; 9,10 <- head 15.
            # qt0's accumulators (2 'sc' + 1 'qr' slot) provide PE cover work
            # on planes 0..7 while the last norm chains complete.
            norm_a(3, nch=4)
            norm_b(3, nch=4)
            issue_r(14)
            norm_a(4, nch=4)
            gproj_qt(0, 0, 8)
            issue_r(15)
            norm_a(5, nch=4)
            norm_b(4, nch=4)
            gproj_qt(0, 8, 9)
            norm_b(5, nch=4)
            gproj_qt(0, 9, NPL)
            for qt in range(1, SH // 128):
                gproj_qt(qt, 0, NPL)

    _ldw_peephole(nc)
    if legalize:
        _legalize_waits(nc, mybir)
    return nc


def _host_prep(Wq, bq, Wk, bk, Wv, bv, Wo, bo):
    """Weight-side host prep (shared by all cores)."""
    bf = ml_dtypes.bfloat16
    Wq, bq = np.asarray(Wq, np.float32), np.asarray(bq, np.float32)
    Wk, bk = np.asarray(Wk, np.float32), np.asarray(bk, np.float32)
    Wv, bv = np.asarray(Wv, np.float32), np.asarray(bv, np.float32)
    Wo, bo = np.asarray(Wo, np.float32), np.asarray(bo, np.float32)

    # mt[d1, h, d2] = (Wq_aug @ Wk_aug^T)[d1, d2], *_aug = [W^T; b] (81, 80)
    wq_aug = np.concatenate([Wq.transpose(0, 2, 1), bq[:, None, :]], 1)  # [H,81,80]
    wk_aug = np.concatenate([Wk.transpose(0, 2, 1), bk[:, None, :]], 1)
    mt = np.einsum("hde,hfe->dhf", wq_aug, wk_aug)  # [81, H, 81]
    mt = np.ascontiguousarray(mt).astype(bf)

    # G_h[d, o] = sum_e Wv_aug[d, e] Wo[o, 80h+e]; row d=0 is the bias row
    # (ones col of xs), bo folded into head 0's row 0.
    wv_aug = np.concatenate([bv[:, None, :], Wv.transpose(0, 2, 1)], 1)  # [H,81,80]
    wo_blocks = Wo.reshape(D, H, DK).transpose(1, 2, 0)  # [H, 80, D]
    g_flat = np.einsum("hde,heo->hdo", wv_aug, wo_blocks).reshape(NJ, D)
    g_flat[0] += bo
    g_pad = np.concatenate([g_flat, np.zeros((128 * NPL - NJ, D), np.float32)])
    g = np.ascontiguousarray(
        g_pad.reshape(NPL, 128, D).transpose(1, 0, 2)
    ).astype(bf)
    return mt, g


def _host_x(src_b, qlo):
    """Per-core activation prep: shuffle channels, roll queries to front,
    emit d-major (xh, ones row last) and s-major (xs, ones col first)."""
    bf = ml_dtypes.bfloat16
    sh = np.asarray(src_b, np.float32).reshape(S, G, D // G)
    sh = sh.transpose(0, 2, 1).reshape(S, D)  # channel shuffle
    xr = np.roll(sh, -qlo, axis=0)
    xh = np.concatenate(
        [xr.reshape(S, H, DK).transpose(2, 1, 0), np.ones((1, H, S), np.float32)]
    )  # [81, H, S]
    xs = np.concatenate(
        [
            np.ones((128, H, NT, 1), np.float32),
            xr.reshape(NT, 128, H, DK).transpose(1, 2, 0, 3),
        ],
        axis=3,
    )  # [128, H, NT, 81]
    return np.ascontiguousarray(xh).astype(bf), np.ascontiguousarray(xs).astype(bf)


def make_in_maps(inputs):
    src = np.asarray(inputs["src"], np.float32)
    mt, g = _host_prep(
        inputs["Wq"], inputs["bq"], inputs["Wk"], inputs["bk"],
        inputs["Wv"], inputs["bv"], inputs["Wo"], inputs["bo"],
    )
    in_maps = []
    for i in range(N_CORES):
        b, qlo = i // 2, (i % 2) * SH
        xh, xs = _host_x(src[b], qlo)
        in_maps.append({"xh": xh, "xs": xs, "mt": mt, "g": g})
    return in_maps


def kernel(**inputs):
    from concourse.bass_utils import run_bass_kernel_spmd

    if "nc" not in _BUILT:
        _BUILT["nc"] = _build()
    nc = _BUILT["nc"]

    in_maps = make_in_maps(inputs)
    res = run_bass_kernel_spmd(nc, in_maps, core_ids=list(range(N_CORES)))

    out = np.empty((B, S, D), np.float32)
    for i in range(N_CORES):
        b, qlo = i // 2, (i % 2) * SH
        out[b, qlo : qlo + SH] = res.results[i]["out"]
    return out


# revision 49
# speedup vs baseline: 1.0895x; 1.0895x over previous
"""Trainium2 Bass kernel for nn_MultiHeadAttention_60816736911814.

Reference semantics (all derived from `src`; `k`/`v` args ignored):
  x  = channel_shuffle(src)          # [B,S,G,C]->[B,S,C,G] flatten, G=5
  xh = split_heads(x)                # [B,H,S,dk], H=16, dk=80
  q/k/v = per-head Linear(dk,dk)     # weights [H,dk,dk] + bias
  attn  = softmax(q k^T / sqrt(dk)) v
  out   = concat(attn) @ Wo^T + bo   # Wo [D,D], D=1280

Sharding (8 cores, no collectives): core i handles batch b=i//2 and query
rows [512*(i%2), +512). Each core gets src[b] ROLLED so its query rows are
rows 0..511 (key order is irrelevant to softmax+sum); all cores run an
identical program and the full output is a pure concatenation.

Device-side algebraic restructuring (all folds are host-side, layout-only
or tiny weight-matrix products):
 - K projection eliminated: scores = x~_k^T Mt x~_q with Mt = Wq_aug @
   Wk_aug^T per head (x~ = [x;1] handles both biases). One projection
   (qq = Mt^T x~_q) replaces q AND k projections.
 - V projection and output projection fused: out = sum_h Rn_h^T G_h where
   R_h = XS_h P_h ([1+dk, SH], row 0 = Z = softmax denominator via the
   ones-column of XS), Rn = R * bcast(1/Z), and G_h = Wv_aug_h @
   Wo_h_block^T (+ bo folded into head 0's row 0, whose Rn value is 1).
 - Channel shuffle / head split / transposes are folded into the host-side
   layouts of xh (d-major) and xs (s-major): no on-device transposes.

Engine schedule: ACT does only the 48 exp instructions (scores land in
2/3-bank fused PSUM tiles so each Exp covers 1536/1024 columns); casts run
on GpSimd, normalization mul + reciprocal on DVE, 1/Z partition-broadcast
via SBUF->SBUF DMA. The PE stream is software-pipelined (scores(h) ->
qq(h+1) -> R(h-1)) so the Tensor engine stays continuously busy (max
p-state clock); qq and R share one rotating 2-slot PSUM tag so everything
fits the 8 PSUM banks alongside the 6 score banks.
"""

import numpy as np
import ml_dtypes

B, S, D = 4, 1024, 1280
H, DK, G = 16, 80, 5
N_CORES = 8
SH = S // 2  # 512 query rows per core
SCALE = 1.0 / float(np.sqrt(DK))
NT = S // 128  # 8 k-tiles
DA = DK + 1  # 81: augmented channel dim
NJ = H * DA  # 1296 packed (h,d) rows
NPL = (NJ + 127) // 128  # 11 planes
# per-head score fusion: k-tiles per fused PSUM tile / exp instruction
FUSE = [3, 3, 2]

_BUILT = {}


def _legalize_waits(nc, mybir):
    """This walrus build allows 1 sync-wait per instruction (2 on
    EventSemaphore). Tile can emit more; split overflow waits onto
    injected same-engine NoOp carriers placed just before the
    instruction (engines run their stream in order -> AND semantics)."""
    n_fix = 0
    for f in nc.m.functions:
        for blk in f.blocks:
            out = []
            changed = False
            for inst in blk.instructions:
                cap = 2 if type(inst).__name__ == "InstEventSemaphore" else 1
                si = inst.sync_info
                if si is not None and si.on_wait and len(si.on_wait) > cap:
                    waits = list(si.on_wait)
                    for w in waits[:-cap]:
                        nop = mybir.InstNoOp(name=f"I-waitfix-{n_fix}")
                        n_fix += 1
                        nop.engine = inst.engine
                        nop.sync_info = mybir.SyncInfo(on_wait=[w], on_update=[])
                        out.append(nop)
                    inst.sync_info = mybir.SyncInfo(
                        on_wait=waits[-cap:], on_update=list(si.on_update)
                    )
                    changed = True
                out.append(inst)
            if changed:
                try:
                    blk.instructions = out
                except Exception:
                    blk.instructions.clear()
                    blk.instructions.extend(out)
    return n_fix


def _ldw_peephole(nc):
    """Drop redundant weight reloads: a matmul whose stationary operand is
    byte-identical to the immediately preceding PE matmul's keeps the loaded
    weights (ldweights=False). Only Gproj's 3-way ocut reuse and the preheat
    chain match."""
    n = 0
    for f in nc.m.functions:
        for blk in f.blocks:
            prev = None
            for inst in blk.instructions:
                if type(inst).__name__ != "InstMatmult":
                    continue
                w = inst.ins[1]
                key = (
                    getattr(w, "memref", None),
                    getattr(w, "offset", None),
                    str(getattr(w, "ap", None)),
                    str(inst.is_transpose),
                    str(inst.perf_mode),
                )
                if prev == key:
                    inst.ldweights = False
                    n += 1
                prev = key
    return n


def _build(legalize=True):
    import os as _os
    import concourse.bass as bass
    import concourse.mybir as mybir
    import concourse.tile as tile

    _fuse = FUSE

    f32 = mybir.dt.float32
    bf16 = mybir.dt.bfloat16

    nc = bass.Bass(trn_type="TRN2", target_bir_lowering=False, debug=False)

    # xh[d, h, s]: d-major shuffled x, row 80 = ones
    xh_d = nc.dram_tensor("xh", [DA, H, S], bf16, kind="ExternalInput").ap()
    # xs[p, h, t, j]: s-major shuffled x (k = t*128+p), col j=0 = ones
    xs_d = nc.dram_tensor("xs", [128, H, NT, DA], bf16, kind="ExternalInput").ap()
    # mt[d1, h, d2] = (Wq_aug @ Wk_aug^T)[d1, d2] per head
    mt_d = nc.dram_tensor("mt", [DA, H, DA], bf16, kind="ExternalInput").ap()
    # g[p, pl, o]: packed rows j = 81h + d of G_h = Wv_aug @ Wo_h^T (+bo)
    g_d = nc.dram_tensor("g", [128, NPL, D], bf16, kind="ExternalInput").ap()
    out_d = nc.dram_tensor("out", [SH, D], f32, kind="ExternalOutput").ap()

    with tile.TileContext(nc) as tc:
        with (
            tc.tile_pool(name="const", bufs=1) as const,
            tc.tile_pool(name="big", bufs=1) as big,
            tc.tile_pool(name="et", bufs=6) as etp,
            tc.tile_pool(name="sm", bufs=2) as sm,
            tc.tile_pool(name="ps", bufs=2, space="PSUM") as ps,
        ):
            on2 = const.tile([128, 512], bf16)
            nc.gpsimd.memset(on2, 1.0)

            mt_sb = big.tile([DA, H, DA], bf16)
            xh_sb = big.tile([DA, H, S], bf16)
            xs_sb = big.tile([128, H, NT, DA], bf16)
            g_sb = big.tile([128, NPL, D], bf16)
            ct = big.tile([DA, H, SH], bf16)  # normalized heads, h-major
            ctp = big.tile([128, NPL, SH], bf16)  # packed rows j = 81h+d

            # --- input DMA: demand-driven. Only what iterations 0..3 need is
            # loaded upfront; the rest is prefetched per-head inside the loop
            # so no single bulk transfer blocks startup. ACT issues no DMAs
            # (DMA transfer time occupies the issuing engine). ---
            nc.sync.dma_start(out=mt_sb, in_=mt_d)
            nc.sync.dma_start(out=xh_sb[:, 0:2, :], in_=xh_d[:, 0:2, :])
            nc.gpsimd.dma_start(out=xs_sb[:, 0:2, :, :], in_=xs_d[:, 0:2, :, :])
            nc.sync.dma_start(out=xh_sb[:, 2:4, :], in_=xh_d[:, 2:4, :])

            # --- PE p-state preheat: dummy matmuls so the clock is ramped
            # when real work starts ---
            for i in range(7):
                ph = ps.tile([DA, 512], f32, tag="qr", bufs=2, name="ph")
                nc.tensor.matmul(
                    ph[0:1, :], on2[0:1, 0:1], on2[0:1, :], start=True, stop=True
                )

            qq_sbs = {}
            ets = {}
            r_pss = {}
            hus = {}
            # head groups for softmax normalization; the last groups are
            # small so the final normalization chains are short
            HGROUPS = [
                (0, 1, 2, 3), (4, 5, 6, 7), (8, 9, 10, 11), (12, 13), (14,), (15,),
            ]
            GOF = {h: (gi, k) for gi, g in enumerate(HGROUPS) for k, h in enumerate(g)}
            zgs = {}
            norm_st = {}

            fetched = {("xh", 0), ("xh", 1), ("xh", 2), ("xh", 3),
                       ("xs", 0), ("xs", 1)}

            def prefetch(kind, h):
                if h > H - 1 or (kind, h) in fetched:
                    return
                fetched.add((kind, h))
                if kind == "xh":
                    nc.sync.dma_start(
                        out=xh_sb[:, h : h + 1, :], in_=xh_d[:, h : h + 1, :]
                    )
                else:
                    nc.gpsimd.dma_start(
                        out=xs_sb[:, h : h + 1, :, :], in_=xs_d[:, h : h + 1, :, :]
                    )

            def issue_qq(h):
                prefetch("xh", h + 3)
                prefetch("xs", h + 1)
                qq_ps = ps.tile([DA, 512], f32, tag="qr", bufs=2, name="qq_ps")
                nc.tensor.matmul(
                    qq_ps, mt_sb[:, h, :], xh_sb[:, h, 0:SH], start=True, stop=True
                )
                qq_sb = sm.tile([DA, 512], bf16, tag="qq_sb", bufs=2, name="qq_sb")
                nc.vector.tensor_copy(qq_sb, qq_ps)
                qq_sbs[h] = qq_sb

            def issue_scores(h):
                if 4 <= h <= 8:
                    j = 2 * (h - 4)
                    eng = nc.sync if h % 2 == 0 else nc.gpsimd
                    eng.dma_start(
                        out=g_sb[:, j : j + 2, :], in_=g_d[:, j : j + 2, :]
                    )
                elif h == 9:
                    nc.sync.dma_start(
                        out=g_sb[:, 10:11, :], in_=g_d[:, 10:11, :]
                    )
                qq_sb = qq_sbs.pop(h)
                ets[h] = []
                kt = 0
                for nf in _fuse:
                    sc = ps.tile([128, nf * 512], f32, tag="sc", bufs=2, name="sc")
                    for u in range(nf):
                        nc.tensor.matmul(
                            sc[:, u * 512 : u * 512 + 512],
                            xh_sb[:, h, (kt + u) * 128 : (kt + u) * 128 + 128],
                            qq_sb,
                            start=True,
                            stop=True,
                        )
                    kt += nf
                    et = etp.tile(
                        [128, nf * 512], bf16, tag="et", name="et",
                        bufs=3 * len(_fuse),
                    )
                    nc.scalar.activation(
                        et[:, 0 : nf * 512],
                        sc[:, 0 : nf * 512],
                        mybir.ActivationFunctionType.Exp,
                        scale=SCALE,
                    )
                    ets[h].append(et)

            def issue_r(h):
                r_ps = ps.tile([DA, 512], f32, tag="qr", bufs=2, name="r_ps")
                kt = 0
                for f, nf in enumerate(_fuse):
                    for u in range(nf):
                        nc.tensor.matmul(
                            r_ps,
                            xs_sb[:, h, kt + u, :],
                            ets[h][f][:, u * 512 : u * 512 + 512],
                            start=(kt + u == 0),
                            stop=(kt + u == NT - 1),
                        )
                    kt += nf
                del ets[h]
                gi, k = GOF[h]
                if len(HGROUPS[gi]) > 1:
                    if k == 0:
                        zgs[gi] = sm.tile(
                            [128, 512], f32, tag="zg", bufs=2, name="zg"
                        )
                        nc.gpsimd.memset(zgs[gi], 1.0)
                    nc.scalar.copy(
                        zgs[gi][32 * k : 32 * k + 1, :], r_ps[0:1, :]
                    )
                else:
                    r_pss[h] = r_ps  # single-head group: recip reads PSUM row
                hu = sm.tile([DA, 512], bf16, tag="hu", bufs=8, name="hu")
                nc.vector.tensor_copy(hu, r_ps)
                hus[h] = hu

            def norm_a(gi, nch=1):
                """Reciprocal of Z + broadcast DMAs for group gi. nch>1
                pipelines the chain in q-tile chunks (tail latency)."""
                nr = 32 * (len(HGROUPS[gi]) - 1) + 1
                zr = sm.tile([128, 512], f32, tag="zr", bufs=2, name="zr")
                zin = zgs.pop(gi) if len(HGROUPS[gi]) > 1 else r_pss.pop(
                    HGROUPS[gi][0]
                )
                w = 512 // nch
                for c in range(nch):
                    nc.vector.reciprocal(
                        zr[0:nr, c * w : c * w + w], zin[0:nr, c * w : c * w + w]
                    )
                zrb = sm.tile([128, 512], bf16, tag="zrb", bufs=2, name="zrb")
                for c in range(nch):
                    nc.vector.tensor_copy(
                        zrb[0:nr, c * w : c * w + w], zr[0:nr, c * w : c * w + w]
                    )
                zbcs = []
                for k in range(len(HGROUPS[gi])):
                    zbc = sm.tile([DA, 512], bf16, tag="zbc", bufs=8, name="zbc")
                    # partition-broadcast via zero-stride middle dim: the DMA
                    # reads the same line DA times; split across both DMA
                    # queues (the 81x re-read is bandwidth-bound)
                    h2 = 256 // nch
                    for c in range(nch):
                        for eng, c0 in ((nc.sync, 2 * c * h2), (nc.gpsimd, 2 * c * h2 + h2)):
                            src = zrb[32 * k : 32 * k + 1, c0 : c0 + h2]
                            src_b = bass.AP(
                                src.tensor, src.offset,
                                [[512, 1], [0, DA], [1, h2]],
                            )
                            eng.dma_start(out=zbc[:, c0 : c0 + h2], in_=src_b)
                    zbcs.append(zbc)
                norm_st[gi] = zbcs

            def norm_b(gi, nch=1):
                """Normalize + repack. In-phase groups multiply on Pool
                (all-SBUF, keeps DVE clear); tail groups on DVE chunked."""
                zbcs = norm_st.pop(gi)
                eng_mul = nc.vector.tensor_mul if gi >= 3 else nc.gpsimd.tensor_mul
                w = 512 // nch
                for k, hh in enumerate(HGROUPS[gi]):
                    hu = hus.pop(hh)
                    j0 = DA * hh
                    pl, off = j0 // 128, j0 % 128
                    l1 = min(128 - off, DA)
                    for c in range(nch):
                        s = slice(c * w, c * w + w)
                        eng_mul(ct[:, hh, s], hu[:, s], zbcs[k][:, s])
                        nc.gpsimd.dma_start(
                            out=ctp[off : off + l1, pl, s], in_=ct[0:l1, hh, s]
                        )
                        if l1 < DA:
                            nc.sync.dma_start(
                                out=ctp[0 : DA - l1, pl + 1, s],
                                in_=ct[l1:DA, hh, s],
                            )

            # --- head phase, software pipelined: PE runs scores(h), qq(h+1),
            # R(h-2); normalization trails by group, spread over two
            # iterations (reciprocal is slow on DVE) and issued after the qq
            # cast so it never delays the next head's scores ---
            issue_qq(0)
            a_due, b_due = [], []
            for h in range(H):
                if h >= 2:
                    hh = h - 2
                    issue_r(hh)
                    gi = GOF[hh][0]
                    if hh == HGROUPS[gi][-1] and hh <= 11:
                        a_due.append(gi)
                issue_scores(h)
                if h + 1 < H:
                    issue_qq(h + 1)
                if b_due:
                    norm_b(b_due.pop(0))
                if a_due:
                    gi = a_due.pop(0)
                    norm_a(gi)
                    b_due.append(gi)

            # --- output projection: out[q, o] = sum_j ctp[j, q] g[j, o].
            # Chunk-outer per q-tile: the three output-column accumulators
            # share each ctp stationary load (the ldweights peephole drops
            # the redundant reloads). ---
            OCUTS = [(0, 512), (512, 1024), (1024, 1280)]
            qt_ops = {}

            def gproj_qt(qt, j_lo, j_hi, tags=("sc", "sc", "qr")):
                if qt not in qt_ops:
                    qt_ops[qt] = [
                        ps.tile([128, 512], f32, tag=tags[c], bufs=2, name="op")
                        for c in range(3)
                    ]
                for j in range(j_lo, j_hi):
                    kh = 128 if j < NPL - 1 else NJ - 128 * (NPL - 1)
                    for c, (o0, o1) in enumerate(OCUTS):
                        nc.tensor.matmul(
                            qt_ops[qt][c][:, 0 : o1 - o0],
                            ctp[0:kh, j, qt * 128 : qt * 128 + 128],
                            g_sb[0:kh, j, o0:o1],
                            start=(j == 0),
                            stop=(j == NPL - 1),
                        )
                if j_hi == NPL:
                    for c, (o0, o1) in enumerate(OCUTS):
                        o_sb = sm.tile(
                            [128, 512], f32, tag="osb", bufs=2, name="o_sb"
                        )
                        nc.vector.tensor_copy(
                            o_sb[:, 0 : o1 - o0], qt_ops[qt][c][:, 0 : o1 - o0]
                        )
                        nc.gpsimd.dma_start(
                            out=out_d[qt * 128 : qt * 128 + 128, o0:o1],
                            in_=o_sb[:, 0 : o1 - o0],
                        )
                    del qt_ops[qt]

            # plane deps: 0..7 <- heads <= 13; 8 <- head 14; 9,10 <- head 15.
            # qt0's accumulators (2 'sc' + 1 'qr' slot) provide PE cover work
            # on planes 0..7 while the last norm chains complete.
            norm_a(3)
            norm_b(3)
            issue_r(14)
            norm_a(4, nch=4)
            gproj_qt(0, 0, 8)
            issue_r(15)
            norm_a(5, nch=4)
            norm_b(4, nch=4)
            gproj_qt(0, 8, 9)
            norm_b(5, nch=4)
            gproj_qt(0, 9, NPL)
            for qt in range(1, SH // 128):
                gproj_qt(qt, 0, NPL)

    _ldw_peephole(nc)
    if legalize:
        _legalize_waits(nc, mybir)
    return nc


def _host_prep(Wq, bq, Wk, bk, Wv, bv, Wo, bo):
    """Weight-side host prep (shared by all cores)."""
    bf = ml_dtypes.bfloat16
    Wq, bq = np.asarray(Wq, np.float32), np.asarray(bq, np.float32)
    Wk, bk = np.asarray(Wk, np.float32), np.asarray(bk, np.float32)
    Wv, bv = np.asarray(Wv, np.float32), np.asarray(bv, np.float32)
    Wo, bo = np.asarray(Wo, np.float32), np.asarray(bo, np.float32)

    # mt[d1, h, d2] = (Wq_aug @ Wk_aug^T)[d1, d2], *_aug = [W^T; b] (81, 80)
    wq_aug = np.concatenate([Wq.transpose(0, 2, 1), bq[:, None, :]], 1)  # [H,81,80]
    wk_aug = np.concatenate([Wk.transpose(0, 2, 1), bk[:, None, :]], 1)
    mt = np.einsum("hde,hfe->dhf", wq_aug, wk_aug)  # [81, H, 81]
    mt = np.ascontiguousarray(mt).astype(bf)

    # G_h[d, o] = sum_e Wv_aug[d, e] Wo[o, 80h+e]; row d=0 is the bias row
    # (ones col of xs), bo folded into head 0's row 0.
    wv_aug = np.concatenate([bv[:, None, :], Wv.transpose(0, 2, 1)], 1)  # [H,81,80]
    wo_blocks = Wo.reshape(D, H, DK).transpose(1, 2, 0)  # [H, 80, D]
    g_flat = np.einsum("hde,heo->hdo", wv_aug, wo_blocks).reshape(NJ, D)
    g_flat[0] += bo
    g_pad = np.concatenate([g_flat, np.zeros((128 * NPL - NJ, D), np.float32)])
    g = np.ascontiguousarray(
        g_pad.reshape(NPL, 128, D).transpose(1, 0, 2)
    ).astype(bf)
    return mt, g


def _host_x(src_b, qlo):
    """Per-core activation prep: shuffle channels, roll queries to front,
    emit d-major (xh, ones row last) and s-major (xs, ones col first)."""
    bf = ml_dtypes.bfloat16
    sh = np.asarray(src_b, np.float32).reshape(S, G, D // G)
    sh = sh.transpose(0, 2, 1).reshape(S, D)  # channel shuffle
    xr = np.roll(sh, -qlo, axis=0)
    xh = np.concatenate(
        [xr.reshape(S, H, DK).transpose(2, 1, 0), np.ones((1, H, S), np.float32)]
    )  # [81, H, S]
    xs = np.concatenate(
        [
            np.ones((128, H, NT, 1), np.float32),
            xr.reshape(NT, 128, H, DK).transpose(1, 2, 0, 3),
        ],
        axis=3,
    )  # [128, H, NT, 81]
    return np.ascontiguousarray(xh).astype(bf), np.ascontiguousarray(xs).astype(bf)


def make_in_maps(inputs):
    src = np.asarray(inputs["src"], np.float32)
    mt, g = _host_prep(
        inputs["Wq"], inputs["bq"], inputs["Wk"], inputs["bk"],
        inputs["Wv"], inputs["bv"], inputs["Wo"], inputs["bo"],
    )
    in_maps = []
    for i in range(N_CORES):
        b, qlo = i // 2, (i % 2) * SH
        xh, xs = _host_x(src[b], qlo)
        in_maps.append({"xh": xh, "xs": xs, "mt": mt, "g": g})
    return in_maps


def kernel(**inputs):
    from concourse.bass_utils import run_bass_kernel_spmd

    if "nc" not in _BUILT:
        _BUILT["nc"] = _build()
    nc = _BUILT["nc"]

    in_maps = make_in_maps(inputs)
    res = run_bass_kernel_spmd(nc, in_maps, core_ids=list(range(N_CORES)))

    out = np.empty((B, S, D), np.float32)
    for i in range(N_CORES):
        b, qlo = i // 2, (i % 2) * SH
        out[b, qlo : qlo + SH] = res.results[i]["out"]
    return out
